# revision 1
# baseline (speedup 1.0000x reference)
"""Two-layer GCN encoder on 8 Trainium2 NeuronCores.

Strategy (dst-sharded graph parallelism):
  out = relu(P @ relu((P @ x) @ W1 + b1) @ W2 + b2),  P = D^-1/2 (A+I) D^-1/2
Each core owns a block of 12500 destination nodes. Sparse propagation
(P @ X) is computed per dst-window of 128 nodes as a sequence of PE
matmuls against dynamically built one-hot "selection" matrices; the
per-edge source rows are fetched with the Q7-accelerated dma_gather
(4 SWDGE queues). Self-loops bypass the gather (they read the core's
own contiguous block). The inter-layer activation table is exchanged
with a single on-chip AllGather. Weights are replicated.
"""
import sys
sys.path.insert(0, "/opt/trn_rl_repo")
import heapq
import numpy as np
from contextlib import ExitStack

# ----------------------------------------------------------------- config ---
N = 100000
E = 600000
NC = 8
BPC = N // NC          # 12500 nodes per core
P = 128
F_IN = 128
F_HID = 256
F_OUT = 128
NW = 98                # windows per core (97*128 + 84)
W_LAST = BPC - 97 * P  # 84
NQ = 4                 # index quarters (int16 gather indices < 25000)
QS = N // NQ           # 25000
CQW = 3                # gathered chunks per (window, quarter)
SGW = 6                # windows per super-group (gather-call granularity)
SGS = [SGW] * (NW // SGW) + ([NW % SGW] if NW % SGW else [])   # [6]*16 + [2]
NCH_G = NW * NQ * CQW  # gathered chunks per core per layer: 1176
NCH = NCH_G + NW       # + self chunks: 1274
BLK_PAD = NW * P       # x_own/g padded rows: 12544
GEMM_CS = [512] * 24 + [BPC - 24 * 512]   # node chunks for the GEMM phase
NAG = 4                 # split-AllGather blocks; NC*(BPC//NAG) == QS
B4 = BPC // NAG
assert NC * B4 == QS

assert sum(SGS) == NW


def _wsize(w):
    return P if w < NW - 1 else W_LAST


# ------------------------------------------------------- host preprocessing ---
def preprocess(x, edge_index, W1, b1, W2, b2):
    s_old = np.asarray(edge_index[0], dtype=np.int64)
    d_old = np.asarray(edge_index[1], dtype=np.int64)
    x = np.asarray(x, dtype=np.float32)

    indeg = np.bincount(d_old, minlength=N)
    deg = (indeg + 1).astype(np.float32)
    dinv = (1.0 / np.sqrt(deg)).astype(np.float32)

    # --- balance real in-degree across the 784 (core, window) bins ---------
    nbins = NC * NW
    slots = np.array([[_wsize(w) for w in range(NW)] for _ in range(NC)]
                     ).reshape(-1)
    order = np.argsort(-indeg, kind="stable")
    heap = [(0, b) for b in range(nbins)]
    heapq.heapify(heap)
    bin_nodes = [[] for _ in range(nbins)]
    bin_left = slots.copy()
    old2new = np.empty(N, dtype=np.int64)
    for node in order:
        while True:
            wgt, b = heapq.heappop(heap)
            if bin_left[b] > 0:
                break
        bin_nodes[b].append(node)
        bin_left[b] -= 1
        if bin_left[b] > 0:
            heapq.heappush(heap, (wgt + int(indeg[node]), b))
    perm_old = np.empty(N, dtype=np.int64)  # perm_old[new] = old
    pos = 0
    for c in range(NC):
        for w in range(NW):
            nodes = bin_nodes[c * NW + w]
            assert len(nodes) == _wsize(w)
            for dl, node in enumerate(nodes):
                old2new[node] = pos + dl
            perm_old[pos:pos + len(nodes)] = nodes
            pos += len(nodes)
    assert pos == N

    x_perm = x[perm_old]                                  # [N, F] new order
    dinv_perm = dinv[perm_old].astype(np.float32)

    s_new = old2new[s_old]
    d_new = old2new[d_old]
    norm = (dinv[s_old] * dinv[d_old]).astype(np.float32)

    core = d_new // BPC
    within = d_new % BPC
    win = within // P
    dl = within % P

    NIC = sum(nw * CQW * P // 16 for nw in SGS) * NQ   # idx columns per core

    def pack(q, sloc):
        """Group edges by (core, window, quarter q) and pack slot arrays."""
        key = ((core * NW + win) * NQ + q)
        eorder = np.lexsort((sloc, key))
        key_s = key[eorder]
        grp_start = np.searchsorted(key_s, np.arange(nbins * NQ))
        grp_end = np.searchsorted(key_s, np.arange(nbins * NQ) + 1)
        if (grp_end - grp_start).max() > CQW * P:
            raise RuntimeError(
                f"quarter-group overflow: {(grp_end - grp_start).max()}")
        sloc_s = sloc[eorder]
        dl_s = dl[eorder]
        norm_s = norm[eorder]
        idx_all = np.zeros((NC, 128, NIC), dtype=np.int16)
        dstloc_all = np.zeros((NC, P, NCH), dtype=np.float32)
        norm_all = np.zeros((NC, P, NCH), dtype=np.float32)
        for c in range(NC):
            icol = 0
            gch = 0
            sgbase = 0
            for nw in SGS:
                ni = nw * CQW * P
                for qq in range(NQ):
                    flat = np.zeros(ni, dtype=np.int64)  # pad idx 0 (real row)
                    for wl in range(nw):
                        w = sgbase + wl
                        g = ((c * NW + w) * NQ + qq)
                        a, b = grp_start[g], grp_end[g]
                        n_e = b - a
                        sl = slice(wl * CQW * P, wl * CQW * P + n_e)
                        flat[sl] = sloc_s[a:b]
                        cbase = gch + qq * nw * CQW + wl * CQW
                        pp = np.arange(n_e)
                        dstloc_all[c, pp % P, cbase + pp // P] = dl_s[a:b]
                        norm_all[c, pp % P, cbase + pp // P] = norm_s[a:b]
                    wr = flat.reshape(ni // 16, 16).T.astype(np.int16)
                    idx_all[c, :, icol:icol + ni // 16] = np.tile(wr, (8, 1))
                    icol += ni // 16
                gch += nw * NQ * CQW
                sgbase += nw
            assert icol == NIC and gch == NCH_G and sgbase == NW
            for w in range(NW):
                ws = _wsize(w)
                pp = np.arange(ws)
                dstloc_all[c, pp, NCH_G + w] = pp
                nodes = c * BPC + w * P + pp
                norm_all[c, pp, NCH_G + w] = dinv_perm[nodes] ** 2
        return idx_all, dstloc_all, norm_all

    # layer 1: plain permuted-id quarters over xt
    idx1_all, dst1_all, nrm1_all = pack(s_new // QS,
                                        (s_new % QS).astype(np.int64))
    # layer 2: rank-block-interleaved quarters over the split-AG output:
    # node m sits at x2 row s*QS + c*B4 + r with c=m//BPC, s=(m%BPC)//B4
    c2 = s_new // BPC
    o2 = s_new % BPC
    idx2_all, dst2_all, nrm2_all = pack(o2 // B4, (c2 * B4 + o2 % B4))

    x_perm16 = x_perm.astype(np.float16)
    x_own = np.zeros((NC, BLK_PAD, F_IN), dtype=np.float16)
    x_own[:, :BPC] = x_perm16.reshape(NC, BPC, F_IN)

    in_maps = []
    for c in range(NC):
        in_maps.append({
            "xt": x_perm16,
            "x_own": x_own[c],
            "idx1": idx1_all[c],
            "dst1": dst1_all[c],
            "nrm1": nrm1_all[c],
            "idx2": idx2_all[c],
            "dst2": dst2_all[c],
            "nrm2": nrm2_all[c],
            "W1": np.asarray(W1, dtype=np.float32),
            "b1": np.asarray(b1, dtype=np.float32),
            "W2": np.asarray(W2, dtype=np.float32),
            "b2": np.asarray(b2, dtype=np.float32),
        })
    return in_maps, perm_old


# ------------------------------------------------------------- bass program ---
import os
X2_SPACE = os.environ.get("X2_SPACE", "Shared")


def build_program(repeat=1, parts="1ga2"):
    from concourse import bass, mybir, tile, bacc, library_config

    f32 = mybir.dt.float32
    f16 = mybir.dt.float16
    nc = bacc.Bacc("TRN2", target_bir_lowering=False, num_devices=NC,
                   num_swdge_queues=4)

    NIC = sum(nw * CQW * P // 16 for nw in SGS) * NQ
    xt = nc.dram_tensor("xt", [N, F_IN], f16, kind="ExternalInput")
    x_own = nc.dram_tensor("x_own", [BLK_PAD, F_IN], f16, kind="ExternalInput")
    idx1 = nc.dram_tensor("idx1", [128, NIC], mybir.dt.int16, kind="ExternalInput")
    dst1 = nc.dram_tensor("dst1", [P, NCH], f32, kind="ExternalInput")
    nrm1 = nc.dram_tensor("nrm1", [P, NCH], f32, kind="ExternalInput")
    idx2 = nc.dram_tensor("idx2", [128, NIC], mybir.dt.int16, kind="ExternalInput")
    dst2 = nc.dram_tensor("dst2", [P, NCH], f32, kind="ExternalInput")
    nrm2 = nc.dram_tensor("nrm2", [P, NCH], f32, kind="ExternalInput")
    W1 = nc.dram_tensor("W1", [F_IN, F_HID], f32, kind="ExternalInput")
    b1 = nc.dram_tensor("b1", [F_HID], f32, kind="ExternalInput")
    W2 = nc.dram_tensor("W2", [F_HID, F_OUT], f32, kind="ExternalInput")
    b2 = nc.dram_tensor("b2", [F_OUT], f32, kind="ExternalInput")
    out = nc.dram_tensor("out", [BPC, F_OUT], f32, kind="ExternalOutput")

    iota_np = np.tile(np.arange(P, dtype=np.float32), (P, 1))
    iota_dram = nc.inline_tensor(iota_np, name="iota_const")
    zeros_dram = nc.inline_tensor(np.zeros((BLK_PAD - BPC, F_OUT), np.float16),
                                  name="zeros_pad")

    with tile.TileContext(nc) as tc, ExitStack() as ctx:
        const = ctx.enter_context(tc.tile_pool(name="const", bufs=1))
        zpool = ctx.enter_context(tc.tile_pool(name="zpool", bufs=1))
        gp = ctx.enter_context(tc.tile_pool(name="gp", bufs=10))
        own = ctx.enter_context(tc.tile_pool(name="own", bufs=3))
        sp = ctx.enter_context(tc.tile_pool(name="sp", bufs=16))
        ep = ctx.enter_context(tc.tile_pool(name="ep", bufs=6))
        psw = ctx.enter_context(tc.tile_pool(name="psw", bufs=4, space="PSUM"))
        psh = ctx.enter_context(tc.tile_pool(name="psh", bufs=1, space="PSUM"))
        psg = ctx.enter_context(tc.tile_pool(name="psg", bufs=2, space="PSUM"))
        hp = ctx.enter_context(tc.tile_pool(name="hp", bufs=2))
        dram = ctx.enter_context(tc.tile_pool(name="dram", bufs=1, space="DRAM"))

        nc.gpsimd.load_library(library_config.mlp)

        # ---- constants / parameters into SBUF -----------------------------
        iota_sb = const.tile([P, P], f32)
        nc.sync.dma_start(iota_sb[:], iota_dram[:])
        idx1_sb = const.tile([128, NIC], mybir.dt.int16)
        nc.sync.dma_start(idx1_sb[:], idx1[:])
        dst1_sb = const.tile([P, NCH], f32)
        nc.sync.dma_start(dst1_sb[:], dst1[:])
        nrm1_sb = const.tile([P, NCH], f32)
        nc.sync.dma_start(nrm1_sb[:], nrm1[:])
        idx2_sb = const.tile([128, NIC], mybir.dt.int16)
        nc.sync.dma_start(idx2_sb[:], idx2[:])
        dst2_sb = const.tile([P, NCH], f32)
        nc.sync.dma_start(dst2_sb[:], dst2[:])
        nrm2_sb = const.tile([P, NCH], f32)
        nc.sync.dma_start(nrm2_sb[:], nrm2[:])
        W1_sb = const.tile([P, F_HID], f32)
        nc.sync.dma_start(W1_sb[:], W1[:])
        W2_sb = const.tile([P, F_HID], f32)
        nc.sync.dma_start(W2_sb[:].rearrange("p (k f) -> p k f", k=2),
                          W2[:].rearrange("(k p) f -> p k f", p=P))
        b1_sb = const.tile([P, 2], f32)
        nc.sync.dma_start(b1_sb[:], b1[:].rearrange("(h p) -> p h", p=P))
        b2_row = const.tile([1, F_OUT], f32)
        nc.sync.dma_start(b2_row[:], b2[None, :])
        b2_sb = const.tile([P, F_OUT], f32)
        nc.gpsimd.partition_broadcast(b2_sb[:], b2_row[:])

        # interlayer tables
        g_loc = dram.tile([BLK_PAD, F_OUT], f16)
        zT = zpool.tile([P, NW * P], f32)   # layer-1 z, feature-major

        # -------------------------------------------------------------------
        def spmm(layer, table, own_tab, idx_sb, dst_sb, nrm_sb, gather_only=False):
            """Emit one sparse propagation. layer=1: zT[:, w*P+dl] (feature
            major); layer=2: node-major windows -> bias+relu -> out DMA."""
            icol = 0
            gch = 0
            sgbase = 0
            callno = 0
            for nw in SGS:
                ni = nw * CQW * P
                k = nw * CQW
                gtiles = []
                for qq in range(NQ):
                    G = gp.tile([P, k * P], f16, tag="g")
                    nc.gpsimd.dma_gather(
                        G[:].rearrange("p (k f) -> p k f", k=k),
                        table[qq * QS:(qq + 1) * QS],
                        idx_sb[:, icol:icol + ni // 16],
                        ni, ni, F_IN,
                        single_packet=False,
                        queue_num=callno % 4,
                    )
                    callno += 1
                    icol += ni // 16
                    gtiles.append(G)
                for wl in range(nw):
                    w = sgbase + wl
                    ws = _wsize(w)
                    if gather_only:
                        continue
                    ps = psw.tile([P, P], f32, space="PSUM", tag="zw")
                    nmm = 0
                    for qq in range(NQ):
                        for j in range(CQW):
                            cidx = gch + qq * nw * CQW + wl * CQW + j
                            S = sp.tile([P, P], f16, tag="s")
                            nc.any.tensor_scalar(
                                out=S[:], in0=iota_sb[:],
                                scalar1=dst_sb[:, cidx:cidx + 1],
                                scalar2=nrm_sb[:, cidx:cidx + 1],
                                op0=mybir.AluOpType.is_equal,
                                op1=mybir.AluOpType.mult,
                            )
                            M = gtiles[qq][:, (wl * CQW + j) * P:(wl * CQW + j + 1) * P]
                            if layer == 1:
                                nc.tensor.matmul(ps[:], lhsT=M, rhs=S[:],
                                                 start=(nmm == 0), stop=False)
                            else:
                                nc.tensor.matmul(ps[:], lhsT=S[:], rhs=M,
                                                 start=(nmm == 0), stop=False)
                            nmm += 1
                    # self chunk: own rows of this window
                    cidx = NCH_G + w
                    Mo = own.tile([P, P], f16, tag="own")
                    nc.sync.dma_start(Mo[:], own_tab[w * P:(w + 1) * P, :])
                    S = sp.tile([P, P], f16, tag="s")
                    nc.any.tensor_scalar(
                        out=S[:], in0=iota_sb[:],
                        scalar1=dst_sb[:, cidx:cidx + 1],
                        scalar2=nrm_sb[:, cidx:cidx + 1],
                        op0=mybir.AluOpType.is_equal,
                        op1=mybir.AluOpType.mult,
                    )
                    if layer == 1:
                        nc.tensor.matmul(ps[:], lhsT=Mo[:], rhs=S[:],
                                         start=False, stop=True)
                        nc.any.tensor_copy(zT[:, w * P:(w + 1) * P], ps[:])
                    else:
                        nc.tensor.matmul(ps[:], lhsT=S[:], rhs=Mo[:],
                                         start=False, stop=True)
                        t1 = ep.tile([P, F_OUT], f32, tag="e1")
                        nc.any.tensor_tensor(out=t1[:ws, :], in0=ps[:ws, :],
                                             in1=b2_sb[:ws, :],
                                             op=mybir.AluOpType.add)
                        t2 = ep.tile([P, F_OUT], f32, tag="e2")
                        nc.any.tensor_scalar(out=t2[:ws, :], in0=t1[:ws, :],
                                             scalar1=0.0, scalar2=None,
                                             op0=mybir.AluOpType.max)
                        nc.sync.dma_start(out[w * P:w * P + ws, :], t2[:ws, :])
                gch += nw * NQ * CQW
                sgbase += nw

        def gemm_phase():
            c0 = 0
            for csz in GEMM_CS:
                hts = []
                for h in range(2):
                    ph = psh.tile([P, 512], f32, space="PSUM", tag=f"ph{h}")
                    nc.tensor.matmul(ph[:, :csz],
                                     lhsT=W1_sb[:, h * P:(h + 1) * P],
                                     rhs=zT[:, c0:c0 + csz],
                                     start=True, stop=True)
                    hT = hp.tile([P, 512], f32, tag=f"ht{h}")
                    nc.scalar.activation(hT[:, :csz], ph[:, :csz],
                                         mybir.ActivationFunctionType.Relu,
                                         bias=b1_sb[:, h:h + 1], scale=1.0)
                    hts.append(hT)
                t0 = 0
                while t0 < csz:
                    st = min(P, csz - t0)
                    pg = psg.tile([P, P], f32, space="PSUM", tag="pg")
                    for kk in range(2):
                        nc.tensor.matmul(pg[:st, :],
                                         lhsT=hts[kk][:, t0:t0 + st],
                                         rhs=W2_sb[:, kk * P:(kk + 1) * P],
                                         start=(kk == 0), stop=(kk == 1))
                    gsb = ep.tile([P, P], f16, tag="gsb")
                    nc.any.tensor_copy(gsb[:st, :], pg[:st, :])
                    nc.sync.dma_start(g_loc[c0 + t0:c0 + t0 + st, :], gsb[:st, :])
                    t0 += st
                c0 += csz
            # zero the padded tail rows of g_loc (read by the last self-chunk)
            zpad = ep.tile([BLK_PAD - BPC, F_OUT], f16, tag="zpad")
            nc.sync.dma_start(zpad[:], zeros_dram[:])
            nc.sync.dma_start(g_loc[BPC:BLK_PAD, :], zpad[:])

        for _rep in range(repeat):
            # layer 1: z1 = P x (feature-major), then g = relu(z1 W1 + b1) W2
            if "1" in parts:
                spmm(1, xt, x_own, idx1_sb, dst1_sb, nrm1_sb,
                     gather_only=("o" in parts))
            if "g" in parts:
                gemm_phase()
            # allgather g -> x2s: the collective writes the ExternalInput-
            # backed buffer directly ("E" mode) or bounces through a Shared
            # internal tile ("1ga2" default probes)
            if True:
                space = "Local" if ("L" in parts or "S" in parts) else X2_SPACE
                tag = "x2L" if space == "Local" else f"x2_{_rep}"
                x2_full = dram.tile([N, F_OUT], f16, addr_space=space, tag=tag)
                if "a" in parts:
                    if "S" in parts:
                        # split AG: 4 collectives, one per node sub-block; each
                        # fires as soon as the gemm wrote those g rows
                        B4 = BPC // NAG
                        for s in range(NAG):
                            nc.gpsimd.collective_compute(
                                "AllGather", mybir.AluOpType.bypass,
                                replica_groups=[list(range(NC))],
                                ins=[g_loc[s * B4:(s + 1) * B4, :].opt()],
                                outs=[x2_full[s * NC * B4:(s + 1) * NC * B4, :].opt()],
                            )
                    else:
                        nc.gpsimd.collective_compute(
                            "AllGather", mybir.AluOpType.bypass,
                            replica_groups=[list(range(NC))],
                            ins=[g_loc[0:BPC, :].opt()], outs=[x2_full[:].opt()],
                        )
                if "2" in parts:
                    if "S" in parts:
                        spmm(2, x2_full, g_loc, idx2_sb, dst2_sb, nrm2_sb)
                    else:
                        spmm(2, x2_full, g_loc, idx1_sb, dst1_sb, nrm1_sb)
                elif "x" in parts:
                    # timing probe: layer-2 gathers from the input table
                    spmm(2, xt, g_loc, idx1_sb, dst1_sb, nrm1_sb)

    nc.finalize()
    return nc


# ------------------------------------------------------------------ runner ---
class SpmdRunner:
    def __init__(self, nc_obj, n_cores):
        import jax
        from jax.sharding import Mesh, PartitionSpec
        from jax.experimental.shard_map import shard_map
        from concourse import mybir, bass2jax
        from concourse.bass2jax import _bass_exec_p, install_neuronx_cc_hook

        install_neuronx_cc_hook()
        self.jax = jax
        self.n_cores = n_cores
        nc = nc_obj
        partition_name = (nc.partition_id_tensor.name
                          if nc.partition_id_tensor else None)

        in_names, out_names, out_avals, zero_outs = [], [], [], []
        for alloc in nc.m.functions[0].allocations:
            if not isinstance(alloc, mybir.MemoryLocationSet):
                continue
            name = alloc.memorylocations[0].name
            if alloc.kind == "ExternalInput":
                if name != partition_name:
                    in_names.append(name)
            elif alloc.kind == "ExternalOutput":
                out_names.append(name)
                shape = tuple(alloc.tensor_shape)
                dtype = mybir.dt.np(alloc.dtype)
                out_avals.append(jax.core.ShapedArray(shape, dtype))
                zero_outs.append(np.zeros(shape, dtype))
        self.n_params = len(in_names)
        self.out_names = out_names
        self.out_avals = out_avals
        self.zero_outs = zero_outs
        self.in_names = list(in_names) + list(out_names)
        if partition_name is not None:
            self.in_names.append(partition_name)
        all_in_names = tuple(self.in_names)
        n_outs = len(out_names)

        def _body(*args):
            operands = list(args)
            if partition_name is not None:
                operands.append(bass2jax.partition_id_tensor())
            outs = _bass_exec_p.bind(
                *operands,
                out_avals=tuple(out_avals),
                in_names=all_in_names,
                out_names=tuple(out_names),
                lowering_input_output_aliases=(),
                sim_require_finite=True,
                sim_require_nnan=True,
                nc=nc,
            )
            return tuple(outs)

        devices = jax.devices()[:n_cores]
        assert len(devices) == n_cores, f"need {n_cores} cores"
        self.mesh = Mesh(np.asarray(devices), ("core",))
        self.pspec = PartitionSpec("core")
        in_specs = (self.pspec,) * (self.n_params + n_outs)
        out_specs = (self.pspec,) * n_outs
        self.fn = jax.jit(
            shard_map(_body, mesh=self.mesh, in_specs=in_specs,
                      out_specs=out_specs, check_rep=False),
            keep_unused=True,
        )

    def stage_inputs(self, in_maps):
        jax = self.jax
        per_core = [[np.asarray(m[name]) for name in self.in_names[:self.n_params]]
                    for m in in_maps]
        args = [np.concatenate([per_core[c][i] for c in range(self.n_cores)],
                               axis=0)
                for i in range(self.n_params)]
        args += [np.zeros((self.n_cores * z.shape[0], *z.shape[1:]), z.dtype)
                 for z in self.zero_outs]
        sharding = jax.sharding.NamedSharding(self.mesh, self.pspec)
        return [jax.device_put(a, sharding) for a in args]

    def run(self, staged):
        out = self.fn(*staged)
        self.jax.block_until_ready(out)
        return out

    def results(self, out_arrs):
        return [
            {name: np.asarray(out_arrs[i]).reshape(
                self.n_cores, *self.out_avals[i].shape)[c]
             for i, name in enumerate(self.out_names)}
            for c in range(self.n_cores)
        ]


_CACHE = {}


def _get_runner():
    if "runner" not in _CACHE:
        nc = build_program()
        _CACHE["runner"] = SpmdRunner(nc, NC)
    return _CACHE["runner"]


def kernel(x, edge_index, W1, b1, W2, b2):
    in_maps, perm_old = preprocess(x, edge_index, W1, b1, W2, b2)
    r = _get_runner()
    staged = r.stage_inputs(in_maps)
    res = r.results(r.run(staged))
    out_cat = np.concatenate([res[c]["out"] for c in range(NC)], axis=0)
    out_full = np.empty((N, F_OUT), dtype=np.float32)
    out_full[perm_old] = out_cat
    return out_full



# revision 18
# speedup vs baseline: 94.4484x; 94.4484x over previous
"""Two-layer GCN encoder on 8 Trainium2 NeuronCores.

Strategy (dst-sharded graph parallelism):
  out = relu(P @ relu((P @ x) @ W1 + b1) @ W2 + b2),  P = D^-1/2 (A+I) D^-1/2
Each core owns a block of 12500 destination nodes. Sparse propagation
(P @ X) is computed feature-major per dst-window of 128 nodes as PE
matmuls against dynamically built one-hot "selection" matrices (f16);
the per-edge source rows are fetched with the Q7-accelerated dma_gather
(4 SWDGE queues). Chunk counts per (window, quarter) are data-dependent
(max over cores of ceil(count/128), SPMD-uniform) to minimise gather
padding. Gathered data, weights, and the inter-layer tables are f16
(PSUM accumulation stays f32). Self-loop blocks live in SBUF (no
per-window DMA); layer-2 bias+relu fuse into one activation op since
features sit on partitions. The inter-layer activation table is
exchanged with one on-chip AllGather (a 4-block split-AG variant
exists behind parts="S" but measured slower). Weights are replicated;
the host un-permutes and transposes the f16 feature-major output.
"""
import sys
sys.path.insert(0, "/opt/trn_rl_repo")
import heapq
import numpy as np
from contextlib import ExitStack

# ----------------------------------------------------------------- config ---
N = 100000
E = 600000
NC = 8
BPC = N // NC          # 12500 nodes per core
P = 128
F_IN = 128
F_HID = 256
F_OUT = 128
NW = 98                # windows per core (97*128 + 84)
W_LAST = BPC - 97 * P  # 84
NQ = 4                 # index quarters (int16 gather indices < 25000)
QS = N // NQ           # 25000
SGW = 6                # windows per super-group (gather-call granularity)
SGS = [SGW] * (NW // SGW) + ([NW % SGW] if NW % SGW else [])   # [6]*16 + [2]
BLK_PAD = NW * P       # x_own padded rows: 12544
GEMM_CS = [512] * 24 + [BPC - 24 * 512]   # node chunks for the GEMM phase
NAG = 4                 # split-AllGather blocks; NC*B4 == QS
B4 = BPC // NAG
assert NC * B4 == QS

assert sum(SGS) == NW


def _wsize(w):
    return P if w < NW - 1 else W_LAST


class Plan:
    """Data-dependent chunk layout, shared by preprocess and build_program.

    kmax[w][q] = chunks for (window w, quarter q), identical on all cores
    (SPMD: one program). chunk_start[w][q] = first global chunk id, ids
    assigned in kernel emission order (sg, q, wl, j). goff[sg][q][wl] =
    column-block offset of window wl inside the (sg, q) gather tile.
    """

    def __init__(self, kmax):
        kmax = np.asarray(kmax, dtype=np.int64)
        assert kmax.shape == (NW, NQ)
        self.kmax = kmax
        self.chunk_start = np.zeros((NW, NQ), dtype=np.int64)
        self.goff = []
        gch = 0
        sgbase = 0
        for nw in SGS:
            sg_off = []
            for q in range(NQ):
                offs = []
                o = 0
                for wl in range(nw):
                    w = sgbase + wl
                    offs.append(o)
                    self.chunk_start[w][q] = gch
                    gch += kmax[w][q]
                    o += kmax[w][q]
                sg_off.append(offs)
            self.goff.append(sg_off)
            sgbase += nw
        self.nch_g = gch                      # gathered chunks per core
        self.nch = gch + NW                   # + self chunks
        self.nic = gch * (P // 16)            # idx columns
        # gather width (chunks) per (sg, q)
        self.sgk = [[int(self.kmax[sgbase:sgbase + nw, q].sum())
                     for q in range(NQ)]
                    for sgbase, nw in zip(np.cumsum([0] + SGS[:-1]), SGS)]

    def key(self):
        return self.kmax.tobytes()


# ------------------------------------------------------- host preprocessing ---
def preprocess(x, edge_index, W1, b1, W2, b2):
    s_old = np.asarray(edge_index[0], dtype=np.int64)
    d_old = np.asarray(edge_index[1], dtype=np.int64)
    x = np.asarray(x, dtype=np.float32)

    indeg = np.bincount(d_old, minlength=N)
    deg = (indeg + 1).astype(np.float32)
    dinv = (1.0 / np.sqrt(deg)).astype(np.float32)

    # --- balance real in-degree across the 784 (core, window) bins ---------
    nbins = NC * NW
    slots = np.array([[_wsize(w) for w in range(NW)] for _ in range(NC)]
                     ).reshape(-1)
    order = np.argsort(-indeg, kind="stable")
    heap = [(0, b) for b in range(nbins)]
    heapq.heapify(heap)
    bin_nodes = [[] for _ in range(nbins)]
    bin_left = slots.copy()
    old2new = np.empty(N, dtype=np.int64)
    for node in order:
        while True:
            wgt, b = heapq.heappop(heap)
            if bin_left[b] > 0:
                break
        bin_nodes[b].append(node)
        bin_left[b] -= 1
        if bin_left[b] > 0:
            heapq.heappush(heap, (wgt + int(indeg[node]), b))
    perm_old = np.empty(N, dtype=np.int64)  # perm_old[new] = old
    pos = 0
    for c in range(NC):
        for w in range(NW):
            nodes = bin_nodes[c * NW + w]
            assert len(nodes) == _wsize(w)
            for dl, node in enumerate(nodes):
                old2new[node] = pos + dl
            perm_old[pos:pos + len(nodes)] = nodes
            pos += len(nodes)
    assert pos == N

    x_perm = x[perm_old]                                  # [N, F] new order
    dinv_perm = dinv[perm_old].astype(np.float32)

    s_new = old2new[s_old]
    d_new = old2new[d_old]
    norm = (dinv[s_old] * dinv[d_old]).astype(np.float32)

    core = d_new // BPC
    within = d_new % BPC
    win = within // P
    dl = within % P

    def pack(q, sloc):
        """Pack edges grouped by (core, window, quarter q) into gather-index
        and chunk slot tables; chunk counts are data-dependent."""
        cnt = np.zeros((NC, NW, NQ), dtype=np.int64)
        np.add.at(cnt, (core, win, q), 1)
        kmax = -(-cnt.max(axis=0) // P)           # [NW, NQ]
        plan = Plan(kmax)

        key = ((core * NW + win) * NQ + q)
        eorder = np.lexsort((sloc, key))
        key_s = key[eorder]
        grp_start = np.searchsorted(key_s, np.arange(nbins * NQ))
        grp_end = np.searchsorted(key_s, np.arange(nbins * NQ) + 1)
        sloc_s = sloc[eorder]
        dl_s = dl[eorder]
        norm_s = norm[eorder]

        NIC, NCH = plan.nic, plan.nch
        idx_all = np.zeros((NC, 128, NIC), dtype=np.int16)
        dst_all = np.zeros((NC, P, NCH), dtype=np.float32)
        nrm_all = np.zeros((NC, P, NCH), dtype=np.float32)
        for c in range(NC):
            icol = 0
            sgbase = 0
            for si, nw in enumerate(SGS):
                for qq in range(NQ):
                    ni = plan.sgk[si][qq] * P
                    if ni == 0:
                        continue
                    flat = np.zeros(ni, dtype=np.int64)  # pad idx 0 (real row)
                    for wl in range(nw):
                        w = sgbase + wl
                        g = ((c * NW + w) * NQ + qq)
                        a, b = grp_start[g], grp_end[g]
                        n_e = b - a
                        assert n_e <= plan.kmax[w][qq] * P
                        base = plan.goff[si][qq][wl] * P
                        flat[base:base + n_e] = sloc_s[a:b]
                        cbase = plan.chunk_start[w][qq]
                        pp = np.arange(n_e)
                        dst_all[c, pp % P, cbase + pp // P] = dl_s[a:b]
                        nrm_all[c, pp % P, cbase + pp // P] = norm_s[a:b]
                    wr = flat.reshape(ni // 16, 16).T.astype(np.int16)
                    idx_all[c, :, icol:icol + ni // 16] = np.tile(wr, (8, 1))
                    icol += ni // 16
                sgbase += nw
            assert icol == NIC
            for w in range(NW):
                ws = _wsize(w)
                pp = np.arange(ws)
                dst_all[c, pp, plan.nch_g + w] = pp
                nodes = c * BPC + w * P + pp
                nrm_all[c, pp, plan.nch_g + w] = dinv_perm[nodes] ** 2
        return plan, idx_all, dst_all, nrm_all

    # layer 1 (and plain-AG layer 2): x2 row of node m is m itself
    plan, idx_all, dst_all, nrm_all = pack(s_new // QS, s_new % QS)
    # split-AG layer 2: x2 row of node m is s*QS + c*B4 + r with c=m//BPC,
    # s=(m%BPC)//B4, r=m%B4 — quarter s IS the AG block index
    c2 = s_new // BPC
    o2 = s_new % BPC
    plan2, idx2_all, dst2_all, nrm2_all = pack(o2 // B4, c2 * B4 + o2 % B4)

    x_perm16 = x_perm.astype(np.float16)
    x_own = np.zeros((NC, BLK_PAD, F_IN), dtype=np.float16)
    x_own[:, :BPC] = x_perm16.reshape(NC, BPC, F_IN)

    in_maps = []
    for c in range(NC):
        in_maps.append({
            "xt": x_perm16,
            "x_own": x_own[c],
            "idx1": idx_all[c],
            "dst1": dst_all[c],
            "nrm1": nrm_all[c],
            "idx2": idx2_all[c],
            "dst2": dst2_all[c],
            "nrm2": nrm2_all[c],
            "W1": np.asarray(W1, dtype=np.float16),
            "b1": np.asarray(b1, dtype=np.float32),
            "W2": np.asarray(W2, dtype=np.float16),
            "b2": np.asarray(b2, dtype=np.float32),
        })
    return in_maps, perm_old, (plan, plan2)


# ------------------------------------------------------------- bass program ---
def build_program(plans, repeat=1, parts="1ga2"):
    plan, plan2 = plans if isinstance(plans, tuple) else (plans, plans)
    from concourse import bass, mybir, tile, bacc, library_config

    f32 = mybir.dt.float32
    f16 = mybir.dt.float16
    nc = bacc.Bacc("TRN2", target_bir_lowering=False, num_devices=NC,
                   num_swdge_queues=4)

    NIC, NCH = plan.nic, plan.nch
    NIC2, NCH2 = plan2.nic, plan2.nch
    xt = nc.dram_tensor("xt", [N, F_IN], f16, kind="ExternalInput")
    x_own = nc.dram_tensor("x_own", [BLK_PAD, F_IN], f16, kind="ExternalInput")
    idx1 = nc.dram_tensor("idx1", [128, NIC], mybir.dt.int16, kind="ExternalInput")
    dst1 = nc.dram_tensor("dst1", [P, NCH], f32, kind="ExternalInput")
    nrm1 = nc.dram_tensor("nrm1", [P, NCH], f32, kind="ExternalInput")
    idx2 = nc.dram_tensor("idx2", [128, NIC2], mybir.dt.int16, kind="ExternalInput")
    dst2 = nc.dram_tensor("dst2", [P, NCH2], f32, kind="ExternalInput")
    nrm2 = nc.dram_tensor("nrm2", [P, NCH2], f32, kind="ExternalInput")
    W1 = nc.dram_tensor("W1", [F_IN, F_HID], f16, kind="ExternalInput")
    b1 = nc.dram_tensor("b1", [F_HID], f32, kind="ExternalInput")
    W2 = nc.dram_tensor("W2", [F_HID, F_OUT], f16, kind="ExternalInput")
    b2 = nc.dram_tensor("b2", [F_OUT], f32, kind="ExternalInput")
    # feature-major output: host transposes back to [BPC, F_OUT]
    out = nc.dram_tensor("out", [F_OUT, BPC], f16, kind="ExternalOutput")

    iota_np = np.tile(np.arange(P, dtype=np.float16), (P, 1))
    iota_dram = nc.inline_tensor(iota_np, name="iota_const")

    with tile.TileContext(nc) as tc, ExitStack() as ctx:
        const = ctx.enter_context(tc.tile_pool(name="const", bufs=1))
        zpool = ctx.enter_context(tc.tile_pool(name="zpool", bufs=1))
        gp = ctx.enter_context(tc.tile_pool(name="gp", bufs=10))
        sp = ctx.enter_context(tc.tile_pool(name="sp", bufs=16))
        ep = ctx.enter_context(tc.tile_pool(name="ep", bufs=6))
        psw = ctx.enter_context(tc.tile_pool(name="psw", bufs=4, space="PSUM"))
        psh = ctx.enter_context(tc.tile_pool(name="psh", bufs=1, space="PSUM"))
        psg = ctx.enter_context(tc.tile_pool(name="psg", bufs=2, space="PSUM"))
        hp = ctx.enter_context(tc.tile_pool(name="hp", bufs=2))
        dram = ctx.enter_context(tc.tile_pool(name="dram", bufs=1, space="DRAM"))

        nc.gpsimd.load_library(library_config.mlp)

        # ---- constants / parameters into SBUF -----------------------------
        iota_sb = const.tile([P, P], f16)
        nc.sync.dma_start(iota_sb[:], iota_dram[:])
        idx1_sb = const.tile([128, NIC], mybir.dt.int16)
        nc.sync.dma_start(idx1_sb[:], idx1[:])
        dst1_sb = const.tile([P, NCH], f32)
        nc.sync.dma_start(dst1_sb[:], dst1[:])
        nrm1_sb = const.tile([P, NCH], f32)
        nc.sync.dma_start(nrm1_sb[:], nrm1[:])
        idx2_sb = const.tile([128, NIC2], mybir.dt.int16)
        nc.sync.dma_start(idx2_sb[:], idx2[:])
        dst2_sb = const.tile([P, NCH2], f32)
        nc.sync.dma_start(dst2_sb[:], dst2[:])
        nrm2_sb = const.tile([P, NCH2], f32)
        nc.sync.dma_start(nrm2_sb[:], nrm2[:])
        W1_sb = const.tile([P, F_HID], f16)
        nc.sync.dma_start(W1_sb[:], W1[:])
        W2_sb = const.tile([P, F_HID], f16)
        nc.sync.dma_start(W2_sb[:].rearrange("p (k f) -> p k f", k=2),
                          W2[:].rearrange("(k p) f -> p k f", p=P))
        b1_sb = const.tile([P, 2], f32)
        nc.sync.dma_start(b1_sb[:], b1[:].rearrange("(h p) -> p h", p=P))
        b2c = const.tile([P, 1], f32)
        nc.sync.dma_start(b2c[:], b2[:, None])

        # self-loop tables resident in SBUF: X_all[p, w*F+f] = x_own[w*P+p, f]
        X_all = const.tile([P, NW * F_IN], f16)
        nc.sync.dma_start(X_all[:].rearrange("p (w f) -> p w f", w=NW),
                          x_own[:].rearrange("(w p) f -> p w f", p=P))
        G_all = const.tile([P, NW * F_OUT], f16)
        # zero the last window's block: its pad partitions (>= W_LAST) are
        # never written by the gemm and must read as 0 in layer-2 self chunks
        nc.vector.memset(G_all[:, (NW - 1) * F_OUT:], 0.0)

        # interlayer tables
        g_loc = dram.tile([BPC, F_OUT], f16)
        zT = zpool.tile([P, NW * P], f16)   # layer-1 z, feature-major

        # -------------------------------------------------------------------
        def spmm(layer, table, idx_sb, dst_sb, nrm_sb, gather_only=False,
                 plan=plan):
            """Emit one sparse propagation. layer=1: zT[:, w*P+dl] (feature
            major); layer=2: node-major windows -> bias+relu -> out DMA."""
            icol = 0
            sgbase = 0
            callno = 0
            for si, nw in enumerate(SGS):
                gtiles = [None] * NQ
                for qq in range(NQ):
                    k = plan.sgk[si][qq]
                    if k == 0:
                        continue
                    ni = k * P
                    G = gp.tile([P, k * P], f16, tag="g")
                    nc.gpsimd.dma_gather(
                        G[:].rearrange("p (k f) -> p k f", k=k),
                        table[qq * QS:(qq + 1) * QS],
                        idx_sb[:, icol:icol + ni // 16],
                        ni, ni, F_IN,
                        single_packet=False,
                        queue_num=callno % 4,
                    )
                    callno += 1
                    icol += ni // 16
                    gtiles[qq] = G
                for wl in range(nw):
                    w = sgbase + wl
                    ws = _wsize(w)
                    if gather_only:
                        continue
                    ps = psw.tile([P, P], f32, space="PSUM", tag="zw")
                    nmm = 0
                    for qq in range(NQ):
                        for j in range(int(plan.kmax[w][qq])):
                            cidx = int(plan.chunk_start[w][qq]) + j
                            cb = plan.goff[si][qq][wl] + j
                            S = sp.tile([P, P], f16, tag="s")
                            nc.any.tensor_scalar(
                                out=S[:], in0=iota_sb[:],
                                scalar1=dst_sb[:, cidx:cidx + 1],
                                scalar2=nrm_sb[:, cidx:cidx + 1],
                                op0=mybir.AluOpType.is_equal,
                                op1=mybir.AluOpType.mult,
                            )
                            M = gtiles[qq][:, cb * P:(cb + 1) * P]
                            nc.tensor.matmul(ps[:], lhsT=M, rhs=S[:],
                                             start=(nmm == 0), stop=False)
                            nmm += 1
                    # self chunk: own rows of this window (SBUF-resident)
                    cidx = plan.nch_g + w
                    own_tab = X_all if layer == 1 else G_all
                    Mo = own_tab[:, w * F_IN:(w + 1) * F_IN]
                    S = sp.tile([P, P], f16, tag="s")
                    nc.any.tensor_scalar(
                        out=S[:], in0=iota_sb[:],
                        scalar1=dst_sb[:, cidx:cidx + 1],
                        scalar2=nrm_sb[:, cidx:cidx + 1],
                        op0=mybir.AluOpType.is_equal,
                        op1=mybir.AluOpType.mult,
                    )
                    nc.tensor.matmul(ps[:], lhsT=Mo, rhs=S[:],
                                     start=(nmm == 0), stop=True)
                    if layer == 1:
                        nc.any.tensor_copy(zT[:, w * P:(w + 1) * P], ps[:])
                    else:
                        # out[f, d] = relu(z2[f, d] + b2[f]) — bias is
                        # per-partition in feature-major, one fused op
                        t2 = ep.tile([P, P], f16, tag="e2")
                        nc.scalar.activation(t2[:, :ws], ps[:, :ws],
                                             mybir.ActivationFunctionType.Relu,
                                             bias=b2c[:], scale=1.0)
                        nc.sync.dma_start(out[:, w * P:w * P + ws], t2[:, :ws])
                sgbase += nw

        def gemm_phase():
            c0 = 0
            for csz in GEMM_CS:
                hts = []
                for h in range(2):
                    ph = psh.tile([P, 512], f32, space="PSUM", tag=f"ph{h}")
                    nc.tensor.matmul(ph[:, :csz],
                                     lhsT=W1_sb[:, h * P:(h + 1) * P],
                                     rhs=zT[:, c0:c0 + csz],
                                     start=True, stop=True)
                    hT = hp.tile([P, 512], f16, tag=f"ht{h}")
                    nc.scalar.activation(hT[:, :csz], ph[:, :csz],
                                         mybir.ActivationFunctionType.Relu,
                                         bias=b1_sb[:, h:h + 1], scale=1.0)
                    hts.append(hT)
                t0 = 0
                while t0 < csz:
                    st = min(P, csz - t0)
                    pg = psg.tile([P, P], f32, space="PSUM", tag="pg")
                    for kk in range(2):
                        nc.tensor.matmul(pg[:st, :],
                                         lhsT=hts[kk][:, t0:t0 + st],
                                         rhs=W2_sb[:, kk * P:(kk + 1) * P],
                                         start=(kk == 0), stop=(kk == 1))
                    # g window tile: node-major block of 128 -> SBUF + DRAM
                    wg = (c0 + t0) // P
                    nc.any.tensor_copy(G_all[:st, wg * F_OUT:(wg + 1) * F_OUT],
                                       pg[:st, :])
                    nc.sync.dma_start(g_loc[c0 + t0:c0 + t0 + st, :],
                                      G_all[:st, wg * F_OUT:(wg + 1) * F_OUT])
                    t0 += st
                c0 += csz

        if "g" in parts and "1" not in parts:
            # timing probes without layer 1: give zT a writer
            nc.vector.memset(zT[:], 0.0)

        for _rep in range(repeat):
            # layer 1: z1 = P x (feature-major), then g = relu(z1 W1 + b1) W2
            if "1" in parts:
                spmm(1, xt, idx1_sb, dst1_sb, nrm1_sb,
                     gather_only=("o" in parts))
            if "g" in parts:
                gemm_phase()
            # allgather g -> x2: layer 2 gathers straight out of it
            if "a" in parts or "2" in parts:
                space = "Local" if ("L" in parts or "S" in parts) else "Shared"
                x2_full = dram.tile([N, F_OUT], f16, addr_space=space,
                                    tag=f"x2_{_rep}" if space != "Local" else "x2L")
                if "a" in parts:
                    if "S" in parts:
                        # split AG: 4 block collectives; block s lands in
                        # x2 rows [s*QS, (s+1)*QS) = layer-2 quarter s, so
                        # quarter-s gathers start as soon as block s lands
                        for s in range(NAG):
                            nc.gpsimd.collective_compute(
                                "AllGather", mybir.AluOpType.bypass,
                                replica_groups=[list(range(NC))],
                                ins=[g_loc[s * B4:(s + 1) * B4, :].opt()],
                                outs=[x2_full[s * QS:(s + 1) * QS, :].opt()],
                            )
                    else:
                        nc.gpsimd.collective_compute(
                            "AllGather", mybir.AluOpType.bypass,
                            replica_groups=[list(range(NC))],
                            ins=[g_loc[0:BPC, :].opt()], outs=[x2_full[:].opt()],
                        )
                if "2" in parts:
                    if "S" in parts:
                        spmm(2, x2_full, idx2_sb, dst2_sb, nrm2_sb, plan=plan2)
                    else:
                        tab2 = xt if "x" in parts else x2_full
                        spmm(2, tab2, idx1_sb, dst1_sb, nrm1_sb)

    nc.finalize()
    return nc


# ------------------------------------------------------------------ runner ---
class SpmdRunner:
    def __init__(self, nc_obj, n_cores):
        import jax
        from jax.sharding import Mesh, PartitionSpec
        from jax.experimental.shard_map import shard_map
        from concourse import mybir, bass2jax
        from concourse.bass2jax import _bass_exec_p, install_neuronx_cc_hook

        install_neuronx_cc_hook()
        self.jax = jax
        self.n_cores = n_cores
        nc = nc_obj
        partition_name = (nc.partition_id_tensor.name
                          if nc.partition_id_tensor else None)

        in_names, out_names, out_avals, zero_outs = [], [], [], []
        for alloc in nc.m.functions[0].allocations:
            if not isinstance(alloc, mybir.MemoryLocationSet):
                continue
            name = alloc.memorylocations[0].name
            if alloc.kind == "ExternalInput":
                if name != partition_name:
                    in_names.append(name)
            elif alloc.kind == "ExternalOutput":
                out_names.append(name)
                shape = tuple(alloc.tensor_shape)
                dtype = mybir.dt.np(alloc.dtype)
                out_avals.append(jax.core.ShapedArray(shape, dtype))
                zero_outs.append(np.zeros(shape, dtype))
        self.n_params = len(in_names)
        self.out_names = out_names
        self.out_avals = out_avals
        self.zero_outs = zero_outs
        self.in_names = list(in_names) + list(out_names)
        if partition_name is not None:
            self.in_names.append(partition_name)
        all_in_names = tuple(self.in_names)
        n_outs = len(out_names)

        def _body(*args):
            operands = list(args)
            if partition_name is not None:
                operands.append(bass2jax.partition_id_tensor())
            outs = _bass_exec_p.bind(
                *operands,
                out_avals=tuple(out_avals),
                in_names=all_in_names,
                out_names=tuple(out_names),
                lowering_input_output_aliases=(),
                sim_require_finite=True,
                sim_require_nnan=True,
                nc=nc,
            )
            return tuple(outs)

        devices = jax.devices()[:n_cores]
        assert len(devices) == n_cores, f"need {n_cores} cores"
        self.mesh = Mesh(np.asarray(devices), ("core",))
        self.pspec = PartitionSpec("core")
        in_specs = (self.pspec,) * (self.n_params + n_outs)
        out_specs = (self.pspec,) * n_outs
        self.fn = jax.jit(
            shard_map(_body, mesh=self.mesh, in_specs=in_specs,
                      out_specs=out_specs, check_rep=False),
            keep_unused=True,
        )

    def stage_inputs(self, in_maps):
        jax = self.jax
        per_core = [[np.asarray(m[name]) for name in self.in_names[:self.n_params]]
                    for m in in_maps]
        args = [np.concatenate([per_core[c][i] for c in range(self.n_cores)],
                               axis=0)
                for i in range(self.n_params)]
        args += [np.zeros((self.n_cores * z.shape[0], *z.shape[1:]), z.dtype)
                 for z in self.zero_outs]
        sharding = jax.sharding.NamedSharding(self.mesh, self.pspec)
        return [jax.device_put(a, sharding) for a in args]

    def run(self, staged):
        out = self.fn(*staged)
        self.jax.block_until_ready(out)
        return out

    def results(self, out_arrs):
        return [
            {name: np.asarray(out_arrs[i]).reshape(
                self.n_cores, *self.out_avals[i].shape)[c]
             for i, name in enumerate(self.out_names)}
            for c in range(self.n_cores)
        ]


_CACHE = {}


PARTS = "1ga2"


def _get_runner(plans):
    key = plans[0].key() + plans[1].key() + PARTS.encode()
    if _CACHE.get("key") != key:
        nc = build_program(plans, parts=PARTS)
        _CACHE["runner"] = SpmdRunner(nc, NC)
        _CACHE["key"] = key
    return _CACHE["runner"]


def kernel(x, edge_index, W1, b1, W2, b2):
    in_maps, perm_old, plans = preprocess(x, edge_index, W1, b1, W2, b2)
    r = _get_runner(plans)
    staged = r.stage_inputs(in_maps)
    res = r.results(r.run(staged))
    out_cat = np.concatenate([res[c]["out"].T for c in range(NC)], axis=0)
    out_full = np.empty((N, F_OUT), dtype=np.float32)
    out_full[perm_old] = out_cat.astype(np.float32)
    return out_full
